# revision 1
# baseline (speedup 1.0000x reference)
"""Causal self-attention (B=4, T=2048, C=1024, H=16) on 8 TRN2 NeuronCores.

Sharding: tensor-parallel over heads. Each core owns 2 heads: it computes
qkv^T for its heads (w_attn column shard), full causal attention for those
heads, and a partial c_proj product (w_proj row shard). The 8 partial
[B*T, C] outputs are summed on the host (the all-reduce of the TP scheme).

Device layout (per core, S^T formulation so softmax reduces on the free axis
via a PE ones-trick, and no max-subtraction — scores are bounded ~N(0,0.4^2)):
  per batch b (pipelined): qkv^T = w_shard^T @ x^T for b's rows;
  per (b, head): S^T tiles = k @ q^T (f32r); P^T = exp(S^T) in bf16
  (causal-sliced); PV^T in bf16 with an appended ones row gives out^T and
  denominators; normalize via reciprocal + K=1 broadcast matmul; then
  y_partial(b) = attn_out @ w_proj_shard (two K=64 f32r matmuls per tile).

Matmuls run in float32r (full-rate fp32 mode, ~12-bit mantissa) except PV
(bf16 probabilities/values).
"""

import sys

for _p in (
    "/opt/trn_rl_repo",
    "/root/.axon_site/_ro/trn_rl_repo",
):
    if _p not in sys.path:
        sys.path.append(_p)

import numpy as np
import concourse.bacc as bacc
import concourse.mybir as mybir
import concourse.tile as tile
from concourse.bass_utils import run_bass_kernel_spmd
from concourse.masks import make_identity, make_upper_triangular

B, T, C, H = 4, 2048, 1024, 16
BT = B * T            # 8192
HS = C // H           # 64
NCORES = 8
HPC = H // NCORES     # heads per core = 2
MQKV = 3 * HPC * HS   # 384 qkv columns per core
NBB = T // 512        # 4 row blocks per batch
KT = C // 128         # 8 contraction tiles
TTK = T // 128        # 16 tk tiles per sequence
TQB = T // 512        # 4 tq blocks per sequence

f32 = mybir.dt.float32
f32r = mybir.dt.float32r
bf16 = mybir.dt.bfloat16
EXPF = mybir.ActivationFunctionType.Exp
IDENTF = mybir.ActivationFunctionType.Identity
PSUM = "PSUM"

import os
QKV_COPY_DVE = os.environ.get("K_QKV_DVE", "1") == "1"
PACK_PROJ = os.environ.get("K_PACK", "0") == "1"
LOOP_N = int(os.environ.get("K_LOOP", "0"))  # >0: wrap body in a HW loop
ST_BUFS = int(os.environ.get("K_ST", "2"))
MISC_BUFS = int(os.environ.get("K_MISC", "2"))
PP_BUFS = int(os.environ.get("K_PP", "4"))
XT_BUFS = int(os.environ.get("K_XT", "2"))
PH = int(os.environ.get("K_PH", "3"))  # 1=qkv+proj, 2=qkv+attn, 3=all


def build_nc():
    nc = bacc.Bacc("TRN2", target_bir_lowering=False, debug=False, num_devices=NCORES)
    xT_d = nc.dram_tensor("xT", [C, BT], f32r, kind="ExternalInput")
    wqkv_d = nc.dram_tensor("wqkv", [C, MQKV], f32r, kind="ExternalInput")
    bqkv_d = nc.dram_tensor("bqkv", [MQKV], f32, kind="ExternalInput")
    wp_d = nc.dram_tensor("wp", [2 * HS, C], f32r, kind="ExternalInput")
    y_d = nc.dram_tensor("y", [BT, C], f32, kind="ExternalOutput")

    wq_src = wqkv_d.ap().rearrange("(k p) m -> p k m", p=128)
    xT_src = xT_d.ap().rearrange("(k p) n -> p k n", p=128)

    with tile.TileContext(nc) as tc:
        with tc.tile_pool(name="const", bufs=1) as const, tc.tile_pool(
            name="qkvp", bufs=1
        ) as qkvp, tc.tile_pool(name="attnp", bufs=2) as attnp, tc.tile_pool(
            name="xt", bufs=XT_BUFS
        ) as xtp, tc.tile_pool(name="v65", bufs=1) as v65p, tc.tile_pool(
            name="pp", bufs=PP_BUFS
        ) as ppool, tc.tile_pool(name="sml", bufs=2) as smlp, tc.tile_pool(
            name="ysb", bufs=3
        ) as ysbp, tc.tile_pool(name="ps_st", bufs=ST_BUFS, space=PSUM) as stp, tc.tile_pool(
            name="ps_po", bufs=2, space=PSUM
        ) as pop, tc.tile_pool(name="ps_mm", bufs=2, space=PSUM) as mmp, tc.tile_pool(
            name="ps_misc", bufs=MISC_BUFS, space=PSUM
        ) as miscp:
            # constants
            ident = const.tile([128, 128], f32)
            make_identity(nc, ident)
            ident_r = const.tile([128, 128], f32r)
            nc.vector.tensor_copy(ident_r[:], ident[:])
            tri = const.tile([128, 128], f32)
            make_upper_triangular(nc, tri, val=1.0, diag=True)  # 1 where part<=free
            tri_b = const.tile([128, 128], bf16)
            nc.vector.tensor_copy(tri_b[:], tri[:])
            ones_col = const.tile([128, 1], f32)
            nc.vector.memset(ones_col, 1.0)
            ones_row = const.tile([1, HS], f32)
            nc.vector.memset(ones_row, 1.0)
            ones1 = const.tile([1, HS], f32r)
            nc.vector.tensor_copy(ones1[:], ones_row[:])
            bias_sb = const.tile([128, 3], f32)
            nc.sync.dma_start(bias_sb[:], bqkv_d.ap().rearrange("(m p) -> p m", p=128))
            w_sb = const.tile([128, KT, MQKV], f32r)
            nc.sync.dma_start(w_sb[:], wq_src)
            wp_sb = const.tile([2 * HS, C], f32r)
            nc.sync.dma_start(wp_sb[:], wp_d[:])
            wp1_sb = const.tile([HS, C], f32r)
            nc.sync.dma_start(wp1_sb[:], wp_d[HS:, :])

            ncopy = 0  # alternate psum->sbuf copies between ACT and DVE

            import contextlib
            loop_cm = tc.For_i(0, LOOP_N, 1) if LOOP_N > 0 else contextlib.nullcontext()
            with loop_cm:
              for b in range(B):
                  tb = b * T
                  # ---- qkv^T for batch b: [128, T] per m in (q, k, v) ----
                  attn_pack = attnp.tile(
                      [128, T], f32r, tag="attn_pack", name=f"attn_pack_{b}", bufs=2
                  )
                  attn1_tmp = attnp.tile(
                      [HS, T], f32r, tag="attn1_tmp", name=f"attn1_tmp_{b}", bufs=2
                  )
                  qkvT_b = [
                      qkvp.tile([128, T], f32r, tag=f"qkvT{m}", name=f"qkvT{m}_{b}", bufs=2)
                      for m in range(3)
                  ]
                  for nb in range(NBB):
                      gnb = b * NBB + nb
                      xt = xtp.tile([128, KT, 512], f32r, tag="xt")
                      nc.sync.dma_start(
                          xt[:], xT_src[:, :, gnb * 512 : (gnb + 1) * 512]
                      )
                      for m in range(3):
                          pq = mmp.tile([128, 512], f32, tag="mm")
                          for k in range(KT):
                              nc.tensor.matmul(
                                  pq[:],
                                  w_sb[:, k, 128 * m : 128 * (m + 1)],
                                  xt[:, k, :],
                                  start=(k == 0),
                                  stop=(k == KT - 1),
                              )
                          if QKV_COPY_DVE:
                              nc.vector.tensor_scalar_add(
                                  qkvT_b[m][:, nb * 512 : (nb + 1) * 512],
                                  pq[:],
                                  bias_sb[:, m : m + 1],
                              )
                          else:
                              nc.scalar.activation(
                                  qkvT_b[m][:, nb * 512 : (nb + 1) * 512],
                                  pq[:],
                                  IDENTF,
                                  bias=bias_sb[:, m : m + 1],
                              )

                  # ---- attention for batch b ----
                  for h in range(HPC if PH != 1 else 0):
                      hp = HS * h  # partition offset of this head
                      # v-transpose: vT [64, T] slices -> v65 tiles [128, 65] bf16
                      v65 = []
                      for i in range(TTK):
                          pt = miscp.tile([128, HS], f32r, tag="misc")
                          nc.tensor.transpose(
                              pt[:, 0:HS],
                              qkvT_b[2][hp : hp + HS, 128 * i : 128 * (i + 1)],
                              ident_r[hp : hp + HS, hp : hp + HS],
                          )
                          vt = v65p.tile(
                              [128, HS + 1], bf16, tag="v65", bufs=2 * TTK, name="vt"
                          )
                          nc.vector.tensor_copy(vt[:, 0:HS], pt[:, 0:HS])
                          nc.vector.tensor_copy(vt[:, HS : HS + 1], ones_col[:])
                          v65.append(vt)
                      for tqb in range(TQB):
                          q0 = tqb * 512  # col offset within batch
                          ntk = 4 * (tqb + 1)
                          po = pop.tile([HS + 1, 512], f32, tag="po")
                          for i in range(ntk):
                              vf = max(0, 128 * i - 512 * tqb)
                              svf = min(vf, 256)  # pad S matmul to N>=256 for f32r
                              st = stp.tile([128, 512], f32, tag="st")
                              nc.tensor.matmul(
                                  st[:, svf:512],
                                  qkvT_b[1][hp : hp + HS, 128 * i : 128 * (i + 1)],
                                  qkvT_b[0][hp : hp + HS, q0 + svf : q0 + 512],
                                  start=True,
                                  stop=True,
                              )
                              ptile = ppool.tile([128, 512], bf16, tag="p", name="ptile")
                              nc.scalar.activation(ptile[:, vf:512], st[:, vf:512], EXPF)
                              if 128 * i >= 512 * tqb:  # diagonal tile
                                  nc.vector.tensor_mul(
                                      ptile[:, vf : vf + 128],
                                      ptile[:, vf : vf + 128],
                                      tri_b[:],
                                  )
                              nc.tensor.matmul(
                                  po[:, vf:512],
                                  v65[i][:],
                                  ptile[:, vf:512],
                                  start=(i == 0),
                                  stop=(i == ntk - 1),
                              )
                          recip = smlp.tile([1, 512], f32r, tag="rcp")
                          with nc.allow_low_precision(reason="softmax recip f32r"):
                              nc.vector.reciprocal(recip[:], po[HS : HS + 1, :])
                          pb = miscp.tile([HS, 512], f32, tag="misc", name="pb")
                          nc.tensor.matmul(
                              pb[0:HS, :], ones1[:], recip[:], start=True, stop=True
                          )
                          po_sb = smlp.tile([HS, 512], f32, tag="posb")
                          nc.scalar.copy(po_sb[:], po[0:HS, :])
                          tt_dst = (
                              attn_pack[0:HS, q0 : q0 + 512]
                              if h == 0
                              else attn1_tmp[:, q0 : q0 + 512]
                          )
                          nc.vector.tensor_mul(tt_dst, po_sb[:], pb[0:HS, :])

                  if PACK_PROJ:
                      nc.sync.dma_start(attn_pack[HS:128, :], attn1_tmp[:])

                  # ---- proj for batch b ----
                  if PH == 1:  # attention skipped: give proj valid inputs
                      attn_pack, attn1_tmp = qkvT_b[0], qkvT_b[1][0:HS, :]
                  for t in range(T // 128 if PH != 2 else 0):
                      ty = ysbp.tile([128, C], f32, tag="y")
                      for n in range(2):
                          py = mmp.tile([128, 512], f32, tag="mm", name="py")
                          if PACK_PROJ:
                              nc.tensor.matmul(
                                  py[:],
                                  attn_pack[:, 128 * t : 128 * (t + 1)],
                                  wp_sb[:, 512 * n : 512 * (n + 1)],
                                  start=True,
                                  stop=True,
                              )
                          else:
                              nc.tensor.matmul(
                                  py[:],
                                  attn_pack[0:HS, 128 * t : 128 * (t + 1)],
                                  wp_sb[0:HS, 512 * n : 512 * (n + 1)],
                                  start=True,
                                  stop=False,
                              )
                              nc.tensor.matmul(
                                  py[:],
                                  attn1_tmp[:, 128 * t : 128 * (t + 1)],
                                  wp1_sb[:, 512 * n : 512 * (n + 1)],
                                  start=False,
                                  stop=True,
                              )
                          if ncopy % 2 == 0:
                              nc.scalar.copy(ty[:, 512 * n : 512 * (n + 1)], py[:])
                          else:
                              nc.vector.tensor_copy(ty[:, 512 * n : 512 * (n + 1)], py[:])
                          ncopy += 1
                      nc.scalar.dma_start(y_d[tb + 128 * t : tb + 128 * (t + 1), :], ty[:])

    nc.compile()
    return nc


_NC_CACHE = None


def _get_nc():
    global _NC_CACHE
    if _NC_CACHE is None:
        _NC_CACHE = build_nc()
    return _NC_CACHE


def make_in_maps(x, w_attn, b_attn, w_proj):
    x = np.ascontiguousarray(np.asarray(x, np.float32).reshape(BT, C))
    w_attn = np.asarray(w_attn, np.float32)
    b_attn = np.asarray(b_attn, np.float32)
    w_proj = np.asarray(w_proj, np.float32)
    xT = np.ascontiguousarray(x.T)
    scale = 1.0 / np.sqrt(HS)
    in_maps = []
    for c in range(NCORES):
        h0 = HPC * c
        cs = slice(HS * h0, HS * (h0 + HPC))
        wq = w_attn[:, 0 * C :][:, cs] * scale
        wk = w_attn[:, 1 * C : 2 * C][:, cs]
        wv = w_attn[:, 2 * C : 3 * C][:, cs]
        wqkv = np.ascontiguousarray(np.concatenate([wq, wk, wv], axis=1))
        bq = b_attn[0 * C :][cs] * scale
        bk = b_attn[1 * C : 2 * C][cs]
        bv = b_attn[2 * C : 3 * C][cs]
        bqkv = np.ascontiguousarray(np.concatenate([bq, bk, bv]))
        in_maps.append(
            {
                "xT": xT,
                "wqkv": wqkv,
                "bqkv": bqkv,
                "wp": np.ascontiguousarray(w_proj[128 * c : 128 * (c + 1), :]),
            }
        )
    return in_maps


def run_on_device(in_maps, **kwargs):
    nc = _get_nc()
    return run_bass_kernel_spmd(nc, in_maps, core_ids=list(range(NCORES)), **kwargs)


def kernel(x, w_attn, b_attn, w_proj, b_proj):
    in_maps = make_in_maps(x, w_attn, b_attn, w_proj)
    res = run_on_device(in_maps)
    y = np.zeros((BT, C), np.float32)
    for r in res.results:
        y += r["y"]
    y += np.asarray(b_proj, np.float32)
    return y.reshape(B, T, C)


if __name__ == "__main__":
    rng = np.random.default_rng(0)
    x = rng.standard_normal((B, T, C)).astype(np.float32)
    w_attn = (rng.standard_normal((C, 3 * C)) * 0.02).astype(np.float32)
    b_attn = np.zeros(3 * C, np.float32)
    w_proj = (rng.standard_normal((C, C)) * 0.02).astype(np.float32)
    b_proj = np.zeros(C, np.float32)
    y = kernel(x, w_attn, b_attn, w_proj, b_proj)
    print("out", y.shape, y.dtype, y[0, 0, :4])



# revision 7
# speedup vs baseline: 1.8159x; 1.8159x over previous
"""Causal self-attention (B=4, T=2048, C=1024, H=16) on 8 TRN2 NeuronCores.

Sharding: tensor-parallel over heads. Each core owns 2 heads: it computes
qkv^T for its heads (w_attn column shard), full causal attention for those
heads, and a partial c_proj product (w_proj row shard). The 8 partial
[B*T, C] bf16 outputs are summed on the host (the all-reduce of the TP
scheme).

v2 design notes (vs v1):
- All matmuls run bf16 (f32 PSUM accumulation). Halves LDWEIGHTS time and
  input DMA (x is shipped pre-transposed as bf16).
- PV uses the natural orientation: out[tq,hs+1] = P^T_chunk^T @ v65, with a
  ones column in v65 producing the softmax denominator as a PSUM *column*.
  Normalization is then a per-partition reciprocal ([128,1], fast DVE
  approx) + one tensor_scalar multiply - the PE never waits on softmax
  stats. (v1 kept out^T, needing a [1,512] single-lane DVE reciprocal
  (3.4us!) plus a broadcast matmul sitting in the in-order PE queue; those
  stalls re-throttled the PE HAM clock gate to 1.2 GHz for ~80% of the
  kernel.)
- The normalized [tq, 2*hs] attention tile is PE-transposed once per
  128-token block, making c_proj a single K=128 matmul (v1 did 2x K=64).
- Software pipelining: the attention phase is ACT-bound (exp is a fixed
  (N+352)/1.2ns on the scalar engine). qkv matmuls of batch b+1, v65
  transposes of the next head, and c_proj of the current batch are emitted
  interleaved into the attention instruction stream so the in-order PE
  queue always has ready work and the HAM clock gate stays at 2.4 GHz.
"""

import sys

for _p in (
    "/opt/trn_rl_repo",
    "/root/.axon_site/_ro/trn_rl_repo",
):
    if _p not in sys.path:
        sys.path.append(_p)

from collections import deque

import numpy as np
import ml_dtypes
import concourse.bacc as bacc
import concourse.mybir as mybir
import concourse.tile as tile
from concourse.bass_utils import run_bass_kernel_spmd
from concourse.masks import make_identity, make_upper_triangular

B, T, C, H = 4, 2048, 1024, 16
BT = B * T            # 8192
HS = C // H           # 64
NCORES = 8
HPC = H // NCORES     # heads per core = 2
MQKV = 3 * HPC * HS   # 384 qkv columns per core
NBB = T // 512        # 4 row blocks per batch
KT = C // 128         # 8 contraction tiles
TTK = T // 128        # 16 key chunks per sequence
TQB = T // 512        # 4 tq blocks per sequence

import os
FILL_NS = int(os.environ.get("K_FILL_NS", "260"))  # filler credit per S-tile

f32 = mybir.dt.float32
bf16 = mybir.dt.bfloat16
EXPF = mybir.ActivationFunctionType.Exp


def build_nc():
    nc = bacc.Bacc("TRN2", target_bir_lowering=False, debug=False, num_devices=NCORES)
    xT_d = nc.dram_tensor("xT", [C, BT], bf16, kind="ExternalInput")
    wqkv_d = nc.dram_tensor("wqkv", [C, MQKV], bf16, kind="ExternalInput")
    bqkv_d = nc.dram_tensor("bqkv", [MQKV], f32, kind="ExternalInput")
    wp_d = nc.dram_tensor("wp", [2 * HS, C], bf16, kind="ExternalInput")
    y_d = nc.dram_tensor("y", [BT, C], bf16, kind="ExternalOutput")

    wq_src = wqkv_d.ap().rearrange("(k p) m -> p k m", p=128)
    xT_src = xT_d.ap().rearrange("(k p) n -> p k n", p=128)

    with tile.TileContext(nc) as tc:
        with tc.tile_pool(name="const", bufs=1) as const, tc.tile_pool(
            name="qkvp", bufs=2
        ) as qkvp, tc.tile_pool(name="asb", bufs=1) as asbp, tc.tile_pool(
            name="xt", bufs=2
        ) as xtp, tc.tile_pool(name="v65", bufs=1) as v65p, tc.tile_pool(
            name="pp", bufs=1
        ) as ppool, tc.tile_pool(name="sml", bufs=1) as smlp, tc.tile_pool(
            name="ysb", bufs=3
        ) as ysbp, tc.tile_pool(name="ps_st", bufs=3, space="PSUM") as stp, tc.tile_pool(
            name="ps_po", bufs=2, space="PSUM"
        ) as pop, tc.tile_pool(name="ps_mm", bufs=2, space="PSUM") as mmp, tc.tile_pool(
            name="ps_small", bufs=1, space="PSUM"
        ) as smallp:
            # ---- constants ----
            ident = const.tile([128, 128], f32)
            make_identity(nc, ident)
            ident_b = const.tile([128, 128], bf16)
            nc.vector.tensor_copy(ident_b[:], ident[:])
            tri = const.tile([128, 128], f32)
            make_upper_triangular(nc, tri, val=1.0, diag=True)  # 1 where part<=free
            tri_b = const.tile([128, 128], bf16)
            nc.vector.tensor_copy(tri_b[:], tri[:])
            ones_b = const.tile([128, 1], bf16)
            nc.vector.memset(ones_b, 1.0)
            bias_sb = const.tile([128, 3], f32)
            nc.sync.dma_start(bias_sb[:], bqkv_d.ap().rearrange("(m p) -> p m", p=128))
            w_sb = const.tile([128, KT, MQKV], bf16)
            nc.sync.dma_start(w_sb[:], wq_src)
            wp_sb = const.tile([2 * HS, C], bf16)
            nc.sync.dma_start(wp_sb[:], wp_d[:])

            # ---- per-batch state ----
            qkvT = {}     # b -> [3 tiles [128, T] bf16]
            v65s = {}     # (b, h) -> list of 16 [128, 65] bf16 tiles
            attn_sb = {}  # b -> list of 16 [128, 128] bf16 tiles

            def get_qkvT(b):
                if b not in qkvT:
                    qkvT[b] = [
                        qkvp.tile(
                            [128, T], bf16, tag=f"qkvT{m}", name=f"qkvT{m}_{b}", bufs=2
                        )
                        for m in range(3)
                    ]
                return qkvT[b]

            def qkv_unit(b, nb, m, xt_box):
                """One third (q|k|v cols) of a 512-token qkv^T block."""
                gnb = b * NBB + nb
                if m == 0:
                    xt_box[0] = xtp.tile(
                        [128, KT, 512], bf16, tag="xt", name=f"xt_{b}_{nb}", bufs=2
                    )
                    nc.sync.dma_start(
                        xt_box[0][:], xT_src[:, :, gnb * 512 : (gnb + 1) * 512]
                    )
                xt = xt_box[0]
                q = get_qkvT(b)
                pq = mmp.tile([128, 512], f32, tag="mm", name="pq")
                for k in range(KT):
                    nc.tensor.matmul(
                        pq[:],
                        w_sb[:, k, 128 * m : 128 * (m + 1)],
                        xt[:, k, :],
                        start=(k == 0),
                        stop=(k == KT - 1),
                    )
                nc.vector.tensor_scalar_add(
                    q[m][:, nb * 512 : (nb + 1) * 512], pq[:], bias_sb[:, m : m + 1]
                )

            def v65_prep_unit(b, h, k):
                """Transpose one 128-key chunk of v^T into natural [key, hs]
                layout with an appended ones column (for denominators)."""
                hp = HS * h
                pt = smallp.tile([128, HS], bf16, tag="ps_small", name="pt")
                nc.tensor.transpose(
                    pt[:],
                    get_qkvT(b)[2][hp : hp + HS, 128 * k : 128 * (k + 1)],
                    ident_b[hp : hp + HS, hp : hp + HS],
                )
                vt = v65p.tile([128, HS + 1], bf16, tag="v65", name="vt", bufs=32)
                nc.vector.tensor_copy(vt[:, 0:HS], pt[:])
                nc.vector.tensor_copy(vt[:, HS : HS + 1], ones_b[:])
                v65s.setdefault((b, h), []).append(vt)

            def proj_unit(b, tidx, atile):
                """c_proj for one 128-token block: transpose the normalized
                attention tile (both heads) and apply the K=128 matmul."""
                aT_ps = smallp.tile([128, 128], bf16, tag="ps_small", name="aT_ps")
                nc.tensor.transpose(aT_ps[:], atile[:], ident_b[:])
                aT = smlp.tile([128, 128], bf16, tag="attnT", name="aT", bufs=4)
                nc.vector.tensor_copy(aT[:], aT_ps[:])
                ty = ysbp.tile([128, C], bf16, tag="ty", name="ty", bufs=3)
                for n in range(2):
                    py = mmp.tile([128, 512], f32, tag="mm", name="py")
                    nc.tensor.matmul(
                        py[:],
                        aT[:],
                        wp_sb[:, 512 * n : 512 * (n + 1)],
                        start=True,
                        stop=True,
                    )
                    nc.vector.tensor_copy(ty[:, 512 * n : 512 * (n + 1)], py[:])
                tb = b * T
                nc.scalar.dma_start(
                    y_d[tb + 128 * tidx : tb + 128 * (tidx + 1), :], ty[:]
                )

            def make_fillers(b, h):
                """Optional filler units (credit-paced) for attn(b, h)'s
                ACT-bound stream: 6 qkv units of b+1 per head, plus v65 prep
                for the next (b, h) pair. Each entry is (est_pe_ns, fn)."""
                units = deque()
                if b + 1 < B:
                    if h == 0:
                        make_fillers._boxes = {nb: [None] for nb in range(NBB)}
                        idxs = [(nb, m) for nb in range(2) for m in range(3)]
                    else:
                        idxs = [(nb, m) for nb in range(2, 4) for m in range(3)]
                    xt_boxes = make_fillers._boxes
                    for nb, m in idxs:
                        units.append(
                            (
                                1700,
                                lambda b=b, nb=nb, m=m, box=xt_boxes[nb]: qkv_unit(
                                    b + 1, nb, m, box
                                ),
                            )
                        )
                if h == 0:
                    for k in range(TTK):
                        units.append((100, lambda b=b, k=k: v65_prep_unit(b, 1, k)))
                elif b + 1 < B:
                    for k in range(TTK):
                        units.append(
                            (100, lambda b=b, k=k: v65_prep_unit(b + 1, 0, k))
                        )
                return units

            # pending: mandatory deferred work (PV+normalize groups, then proj
            # units), popped into later S-streams so the in-order PE queue
            # only ever meets long-completed exp results.
            pending = deque()

            def attn_head(b, h):
                hp = HS * h
                q3 = get_qkvT(b)
                v65 = v65s[(b, h)]
                fillers = make_fillers(b, h)
                credit = 0.0

                def pv_norm_unit(tqb, j, ptl, absb, abst):
                    """One tq-128-chunk: PV accumulation over its key chunks
                    (own PSUM bank = own accumulation group), then softmax
                    normalize via the denominator column."""
                    po = pop.tile([128, 65], f32, tag="po", name="po")
                    last = 4 * tqb + j
                    for i in range(last + 1):
                        nc.tensor.matmul(
                            po[:],
                            ptl[i][:, 128 * j : 128 * (j + 1)],
                            v65[i][:, :],
                            start=(i == 0),
                            stop=(i == last),
                        )
                    rc = smlp.tile([128, 1], f32, tag="recip", name="rc", bufs=8)
                    nc.vector.reciprocal_approx_fast(rc[:], po[:, HS : HS + 1])
                    nc.vector.tensor_scalar_mul(
                        abst[:, hp : hp + HS], po[:, 0:HS], rc[:]
                    )

                for tqb in range(TQB):
                    q0 = tqb * 512
                    ntk = 4 * (tqb + 1)
                    ptl = [None] * ntk
                    for i in range(ntk):
                        vf = max(0, 128 * i - 512 * tqb)
                        st = stp.tile([128, 512], f32, tag="st", name="st")
                        nc.tensor.matmul(
                            st[:, vf:512],
                            q3[1][hp : hp + HS, 128 * i : 128 * (i + 1)],
                            q3[0][hp : hp + HS, q0 + vf : q0 + 512],
                            start=True,
                            stop=True,
                        )
                        pt_ = ppool.tile(
                            [128, 512], bf16, tag="p", name="ptile", bufs=36
                        )
                        ptl[i] = pt_
                        nc.scalar.activation(pt_[:, vf:512], st[:, vf:512], EXPF)
                        if i >= 4 * tqb:  # diagonal chunk: causal mask
                            nc.vector.tensor_mul(
                                pt_[:, vf : vf + 128], pt_[:, vf : vf + 128], tri_b[:]
                            )
                        if pending:
                            pending.popleft()()
                            if len(pending) > 6 and pending:
                                pending.popleft()()
                        credit += FILL_NS
                        while fillers and credit >= fillers[0][0]:
                            credit -= fillers[0][0]
                            fillers.popleft()[1]()
                    for j in range(4):
                        pending.append(
                            lambda tqb=tqb, j=j, ptl=ptl, abst=attn_sb[b][
                                4 * tqb + j
                            ]: pv_norm_unit(tqb, j, ptl, None, abst)
                        )
                        if h == 1:
                            pending.append(
                                lambda b=b, tidx=4 * tqb + j, atile=attn_sb[b][
                                    4 * tqb + j
                                ]: proj_unit(b, tidx, atile)
                            )
                # leftover fillers run at head end (cheap; mostly v65 preps)
                while fillers:
                    fillers.popleft()[1]()

            # ---- schedule ----
            # upfront: qkv(0) + v65(0, h0)
            box0 = {nb: [None] for nb in range(NBB)}
            for nb in range(NBB):
                for m in range(3):
                    qkv_unit(0, nb, m, box0[nb])
            for k in range(TTK):
                v65_prep_unit(0, 0, k)

            for b in range(B):
                attn_sb[b] = [
                    asbp.tile(
                        [128, 128], bf16, tag="asb", name=f"attn_{b}_{t}", bufs=32
                    )
                    for t in range(TTK)
                ]
                attn_head(b, 0)
                attn_head(b, 1)
                if b - 1 in qkvT:
                    del qkvT[b - 1]
            while pending:
                pending.popleft()()

    nc.compile()
    return nc


_NC_CACHE = None


def _get_nc():
    global _NC_CACHE
    if _NC_CACHE is None:
        _NC_CACHE = build_nc()
    return _NC_CACHE


def make_in_maps(x, w_attn, b_attn, w_proj):
    x = np.ascontiguousarray(np.asarray(x, np.float32).reshape(BT, C))
    w_attn = np.asarray(w_attn, np.float32)
    b_attn = np.asarray(b_attn, np.float32)
    w_proj = np.asarray(w_proj, np.float32)
    xT = np.ascontiguousarray(x.T.astype(ml_dtypes.bfloat16))
    scale = 1.0 / np.sqrt(HS)
    in_maps = []
    for c in range(NCORES):
        h0 = HPC * c
        cs = slice(HS * h0, HS * (h0 + HPC))
        wq = w_attn[:, 0 * C :][:, cs] * scale
        wk = w_attn[:, 1 * C : 2 * C][:, cs]
        wv = w_attn[:, 2 * C : 3 * C][:, cs]
        wqkv = np.ascontiguousarray(
            np.concatenate([wq, wk, wv], axis=1).astype(ml_dtypes.bfloat16)
        )
        bq = b_attn[0 * C :][cs] * scale
        bk = b_attn[1 * C : 2 * C][cs]
        bv = b_attn[2 * C : 3 * C][cs]
        bqkv = np.ascontiguousarray(np.concatenate([bq, bk, bv]))
        in_maps.append(
            {
                "xT": xT,
                "wqkv": wqkv,
                "bqkv": bqkv,
                "wp": np.ascontiguousarray(
                    w_proj[128 * c : 128 * (c + 1), :].astype(ml_dtypes.bfloat16)
                ),
            }
        )
    return in_maps


def run_on_device(in_maps, **kwargs):
    nc = _get_nc()
    return run_bass_kernel_spmd(nc, in_maps, core_ids=list(range(NCORES)), **kwargs)


def kernel(x, w_attn, b_attn, w_proj, b_proj):
    in_maps = make_in_maps(x, w_attn, b_attn, w_proj)
    res = run_on_device(in_maps)
    y = np.zeros((BT, C), np.float32)
    for r in res.results:
        y += np.asarray(r["y"], dtype=np.float32)
    y += np.asarray(b_proj, np.float32)
    return y.reshape(B, T, C)


if __name__ == "__main__":
    rng = np.random.default_rng(0)
    x = rng.standard_normal((B, T, C)).astype(np.float32)
    w_attn = (rng.standard_normal((C, 3 * C)) * 0.02).astype(np.float32)
    b_attn = np.zeros(3 * C, np.float32)
    w_proj = (rng.standard_normal((C, C)) * 0.02).astype(np.float32)
    b_proj = np.zeros(C, np.float32)
    y = kernel(x, w_attn, b_attn, w_proj, b_proj)
    print("out", y.shape, y.dtype, y[0, 0, :4])
